# revision 1
# baseline (speedup 1.0000x reference)
"""ContMix kernel for TRN2, 8 NeuronCores.

Sharding: (batch b, H-half) -> 8 cores. Each core computes out[b, :, r0:r0+28, :].

Math (per batch b):
  ctx_p = avgpool8x8(ctx)                      [C, 49]
  kf    = Wk @ ctx_p                           [192, 49]
  G     = Wq^T @ kf                            [C, 49]      (so logits = G^T x, q never materialized)
  lg    = G^T @ x                              [49, HW]
  A     = softmax_s(lg)  (no max-sub; |lg| ~ 8)
  dynT  = A^T @ Wwd^T                          [HW, 25]  (pixel-partition layout)
  out[c, n] = sum_j x_patch[c, j, n] * dyn[j, n]
The last step runs on TensorE as banded matmuls: per output-row-pair, a
[128 x 112] banded matrix M (built from dyn via gpsimd local_scatter into
M^T, then fp16 DMA-xbar transpose) contracts against x in pixel-partition
layout (xt, host-pretransposed, zero-padded rows/cols, row stride 64).
"""

import numpy as np

B, C, H, W = 4, 384, 56, 56
KK, S = 5, 7
NCORES = 8
ROWS = H // 2              # 28 rows per core
NPIX = ROWS * W            # 1568
PADR = ROWS + 4            # 32 padded rows
PADW = 64                  # padded width (>= 56 + 4, and 64-aligned)
WSPACE = PADR * PADW       # 2048 padded pixels
NPAIR = ROWS // 2          # 14 output row-pairs
NCHUNK = PADR // 2         # 16 contraction chunks (2 padded rows = 128 partitions)
D2 = C // 2                # 192
NI = 26                    # scatter idxs (25 taps + 1 pad, must be even)
MCOLS = 3 * 128            # 384 = w''-space per pair (6 rows x 64)

_cached = {}


def _build_nc():
    import concourse.tile as tile
    from concourse import bacc, mybir, library_config

    f32, f16, i16 = mybir.dt.float32, mybir.dt.float16, mybir.dt.int16
    nc = bacc.Bacc("TRN2", target_bir_lowering=False, debug=False)

    xn_d = nc.dram_tensor("xn", [C, NPIX], f16, kind="ExternalInput")
    xt_d = nc.dram_tensor("xt", [WSPACE, C], f16, kind="ExternalInput")
    cx_d = nc.dram_tensor("cx", [C, H * W], f32, kind="ExternalInput")
    wq_d = nc.dram_tensor("wq", [D2, C], f32, kind="ExternalInput")
    wkt_d = nc.dram_tensor("wkt", [C, D2], f32, kind="ExternalInput")
    wwdt1_d = nc.dram_tensor("wwdt1", [S * S, NI], f32, kind="ExternalInput")
    sidx_d = nc.dram_tensor("sidx", [2 * W, NI], i16, kind="ExternalInput")
    out_d = nc.dram_tensor("out", [C, NPIX], f32, kind="ExternalOutput")

    HW = H * W
    with tile.TileContext(nc) as tc:
        with (
            tc.tile_pool(name="big", bufs=1) as big,
            tc.tile_pool(name="wrk", bufs=3) as wrk,
            tc.tile_pool(name="mtp", bufs=4) as mtp,
            tc.tile_pool(name="ps", bufs=8, space="PSUM") as ps,
            tc.tile_pool(name="dr", bufs=4, space="DRAM") as dr,
        ):
            # ---------------- input DMAs ----------------
            xt_sb = big.tile([128, NCHUNK, C], f16, tag="xt")
            nc.sync.dma_start(out=xt_sb[:], in_=xt_d[:].rearrange("(t p) c -> p t c", p=128))
            xn_sb = big.tile([128, 3, NPIX], f16, tag="xn")
            nc.sync.dma_start(out=xn_sb[:], in_=xn_d[:].rearrange("(u p) n -> p u n", p=128))
            cx_sb = big.tile([128, 3, HW], f32, tag="cx")
            nc.sync.dma_start(out=cx_sb[:], in_=cx_d[:].rearrange("(u p) n -> p u n", p=128))
            wqA = big.tile([128, C], f32, tag="wqA")
            nc.sync.dma_start(out=wqA[:], in_=wq_d[0:128, :])
            wqB = big.tile([64, C], f32, tag="wqB")
            nc.sync.dma_start(out=wqB[:], in_=wq_d[128:D2, :])
            wkt_sb = big.tile([128, 3, D2], f32, tag="wkt")
            nc.sync.dma_start(out=wkt_sb[:], in_=wkt_d[:].rearrange("(u p) d -> p u d", p=128))
            wwdt1_sb = big.tile([S * S, NI], f32, tag="wwdt1")
            nc.sync.dma_start(out=wwdt1_sb[:], in_=wwdt1_d[:])
            sidx_sb = big.tile([2 * W, NI], i16, tag="sidx")
            nc.sync.dma_start(out=sidx_sb[:], in_=sidx_d[:])

            nc.gpsimd.load_library(library_config.local_scatter)

            # ---------------- pooling (sum; the /64 is folded into wkt) ----------------
            ctx_p = big.tile([128, 3, S * S], f32, tag="ctxp")
            for u in range(3):
                p1 = wrk.tile([128, H * S], f32, tag="p1")
                nc.vector.tensor_reduce(
                    out=p1[:], in_=cx_sb[:, u, :].rearrange("p (h bw j) -> p h bw j", bw=S, j=8),
                    axis=mybir.AxisListType.X, op=mybir.AluOpType.add)
                # p1 layout (h, bw): strides h->7, bw->1; reduce i (stride 7*... rows within bin)
                ap2 = p1[:].rearrange("p (bh i bw) -> p bh i bw", bh=S, i=8)
                ap2 = ap2.rearrange("p bh i bw -> p bh bw i")
                nc.vector.tensor_reduce(
                    out=ctx_p[:, u, :], in_=ap2,
                    axis=mybir.AxisListType.X, op=mybir.AluOpType.add)

            # ---------------- kf = (Wk/64) @ ctx_p : [192, 49] ----------------
            kf_sb = [big.tile([128, S * S], f32, tag="kf0", name="kf0"),
                     big.tile([64, S * S], f32, tag="kf1", name="kf1")]
            for dc, dw in ((0, 128), (1, 64)):
                kf_ps = ps.tile([dw, S * S], f32, tag="ps")
                for u in range(3):
                    nc.tensor.matmul(kf_ps[:], wkt_sb[:, u, dc * 128:dc * 128 + dw],
                                     ctx_p[:, u, :], start=(u == 0), stop=(u == 2))
                nc.vector.tensor_copy(kf_sb[dc][:], kf_ps[:])

            # ---------------- G = Wq^T @ kf : [384, 49], fp16 ----------------
            g_sb = big.tile([128, 3, S * S], f16, tag="g")
            for u in range(3):
                g_ps = ps.tile([128, S * S], f32, tag="ps")
                nc.tensor.matmul(g_ps[:], wqA[:, u * 128:(u + 1) * 128], kf_sb[0][:],
                                 start=True, stop=False)
                nc.tensor.matmul(g_ps[:], wqB[:, u * 128:(u + 1) * 128], kf_sb[1][:],
                                 start=False, stop=True)
                nc.scalar.copy(g_sb[:, u, :], g_ps[:])

            # ---------------- logits + exp : expa [49, NPIX] fp32 ----------------
            expa = big.tile([S * S, NPIX], f32, tag="expa")
            off = 0
            while off < NPIX:
                wdt = min(512, NPIX - off)
                lg = ps.tile([S * S, 512], f32, tag="ps")
                for u in range(3):
                    nc.tensor.matmul(lg[:, 0:wdt], g_sb[:, u, :], xn_sb[:, u, off:off + wdt],
                                     start=(u == 0), stop=(u == 2))
                nc.scalar.activation(expa[:, off:off + wdt], lg[:, 0:wdt],
                                     mybir.ActivationFunctionType.Exp)
                off += wdt

            # ---------------- per-pair: dyn -> scatter -> M (via DRAM xbar transpose) ----
            m_sb = big.tile([128, NPAIR * 3, 112], f16, tag="m")
            for p0 in range(NPAIR):
                dyn_ps = ps.tile([2 * W, NI], f32, tag="ps")
                nc.tensor.matmul(dyn_ps[:], expa[:, p0 * 112:(p0 + 1) * 112],
                                 wwdt1_sb[:], start=True, stop=True)
                rec = wrk.tile([2 * W, 1], f32, tag="rec")
                nc.vector.reciprocal(rec[:], dyn_ps[:, 25:26])
                d16 = wrk.tile([2 * W, NI], f16, tag="d16")
                nc.vector.tensor_scalar_mul(d16[:], dyn_ps[:], rec[:])
                mt = mtp.tile([2 * W, MCOLS], f16, tag="mt")
                nc.gpsimd.local_scatter(mt[:], d16[:], sidx_sb[:],
                                        channels=2 * W, num_elems=MCOLS, num_idxs=NI)
                mtd = dr.tile([2 * W, MCOLS], f16, tag="mtd")
                nc.sync.dma_start(out=mtd[:], in_=mt[:])
                for t3 in range(3):
                    nc.sync.dma_start_transpose(
                        out=m_sb[:, p0 * 3 + t3, :], in_=mtd[:, t3 * 128:(t3 + 1) * 128])

            # ---------------- final banded matmuls ----------------
            out_sb = big.tile([128, 3, NPIX], f32, tag="out")
            ncopy = 0
            for cc in range(3):
                po = {}
                for t in range(NCHUNK):
                    for p0 in range(max(0, t - 2), min(NPAIR, t + 1)):
                        trel = t - p0
                        if trel == 0:
                            po[p0] = ps.tile([128, 112], f32, tag="ps", name=f"po_{cc}_{p0}")
                        nc.tensor.matmul(po[p0][:], xt_sb[:, t, cc * 128:(cc + 1) * 128],
                                         m_sb[:, p0 * 3 + trel, :],
                                         start=(trel == 0), stop=(trel == 2))
                        if trel == 2:
                            dst = out_sb[:, cc, p0 * 112:(p0 + 1) * 112]
                            if ncopy % 2 == 0:
                                nc.vector.tensor_copy(dst, po[p0][:])
                            else:
                                nc.scalar.copy(dst, po[p0][:])
                            ncopy += 1
                            del po[p0]

            nc.sync.dma_start(out=out_d[:].rearrange("(u p) n -> p u n", p=128), in_=out_sb[:])
    nc.finalize()
    return nc


def _static_inputs():
    # scatter index table: pixel p = hl*56 + w ; tap j = 5*di + dj
    sidx = np.full((2 * W, NI), -1, np.int16)
    for hl in range(2):
        for w in range(W):
            for di in range(KK):
                for dj in range(KK):
                    sidx[hl * W + w, 5 * di + dj] = (hl + di) * PADW + w + dj
    return sidx


def _prep(x, ctx, Wq, Wk, Wwd):
    sidx = _static_inputs()
    wkt = (Wk.T / 64.0).astype(np.float32).copy()
    wwdt1 = np.concatenate([Wwd.T, np.ones((S * S, 1), np.float32)], axis=1).astype(np.float32)
    wq = np.ascontiguousarray(Wq.astype(np.float32))
    in_maps = []
    for core in range(NCORES):
        b, half = core // 2, core % 2
        r0 = half * ROWS
        xn = np.ascontiguousarray(x[b, :, r0:r0 + ROWS, :].reshape(C, NPIX)).astype(np.float16)
        xp = np.zeros((PADR, PADW, C), np.float32)
        lo, hi = max(0, r0 - 2), min(H, r0 + ROWS + 2)
        xp[lo - (r0 - 2):hi - (r0 - 2), 2:2 + W, :] = np.transpose(x[b, :, lo:hi, :], (1, 2, 0))
        xt = xp.reshape(WSPACE, C).astype(np.float16)
        cx = np.ascontiguousarray(ctx[b].reshape(C, H * W)).astype(np.float32)
        in_maps.append(dict(xn=xn, xt=xt, cx=cx, wq=wq, wkt=wkt, wwdt1=wwdt1, sidx=sidx))
    return in_maps


def kernel(x, ctx, Wq, Wk, Wwd, _trace=False):
    from concourse.bass_utils import run_bass_kernel_spmd

    x, ctx = np.asarray(x), np.asarray(ctx)
    Wq, Wk, Wwd = np.asarray(Wq), np.asarray(Wk), np.asarray(Wwd)
    if "nc" not in _cached:
        _cached["nc"] = _build_nc()
    in_maps = _prep(x, ctx, Wq, Wk, Wwd)
    res = run_bass_kernel_spmd(_cached["nc"], in_maps, list(range(NCORES)), trace=_trace)
    _cached["last_result"] = res
    out = np.empty((B, C, H, W), np.float32)
    for core in range(NCORES):
        b, half = core // 2, core % 2
        r0 = half * ROWS
        out[b, :, r0:r0 + ROWS, :] = res.results[core]["out"].reshape(C, ROWS, W)
    return out



# revision 5
# speedup vs baseline: 1.9819x; 1.9819x over previous
"""ContMix kernel for TRN2, 8 NeuronCores.

Sharding: (batch b, H-half) -> 8 cores. Each core computes out[b, :, r0:r0+28, :].

Math (per batch b):
  ctx_p = avgpool8x8(ctx)                      [C, 49]   (DVE reduce, f16)
  kf    = (Wk/64) @ ctx_p                      [192, 49]
  G     = Wq^T @ kf                            [C, 49]
  lg    = G^T @ x                              [49, NPIX] (per 224-col chunk)
  expa  = exp(lg)  (no max-sub; |lg| ~ 8.5, e^max ~ 5e3 fits f16)
  dyn   = expa^T @ [Wwd^T | 1]                 [112, 26] per row-pair
  d16   = dyn / dyn[:, 25]  (softmax denom)
  M^T   = local_scatter(d16) on gpsimd         [112, 384]  (banded matrix)
  M     = PE transpose of M^T chunks           [128, 112] x3 per pair
  out[c, n] = banded matmuls: xt[w''-space, c]^T @ M  accumulated over 3 chunks
All DMAs f16; weights packed in one blob; out streamed in 4 column chunks.
"""

import numpy as np

B, C, H, W = 4, 384, 56, 56
KK, S = 5, 7
NCORES = 8
ROWS = H // 2              # 28 rows per core
NPIX = ROWS * W            # 1568
PADR = ROWS + 4            # 32 padded rows
PADW = 64                  # padded width
WSPACE = PADR * PADW       # 2048 padded pixels
NPAIR = ROWS // 2          # 14 output row-pairs
NCHUNK = PADR // 2         # 16 contraction chunks
D2 = C // 2                # 192
NI = 26                    # scatter idxs (25 taps + 1 denom col)
MCOLS = 3 * 128            # 384 = w''-space per pair
HW = H * W                 # 3136
FB = 1370                  # blob cols: wqA 0:384, wqB 384:768, wkt 768:1344, wwdt1 1344:1370

_cached = {}


def _build_nc():
    import concourse.tile as tile
    from concourse import bacc, mybir, library_config, masks

    f32, f16, i16 = mybir.dt.float32, mybir.dt.float16, mybir.dt.int16
    nc = bacc.Bacc("TRN2", target_bir_lowering=False, debug=False)

    blob_d = nc.dram_tensor("blob", [128, FB], f16, kind="ExternalInput")
    sidx_d = nc.dram_tensor("sidx", [2 * W, NI], i16, kind="ExternalInput")
    cx_d = nc.dram_tensor("cx", [C, HW], f16, kind="ExternalInput")
    xn_d = nc.dram_tensor("xn", [C, NPIX], f16, kind="ExternalInput")
    xt_d = nc.dram_tensor("xt", [WSPACE, C], f16, kind="ExternalInput")
    out_d = nc.dram_tensor("out", [C, NPIX], f16, kind="ExternalOutput")

    with tile.TileContext(nc) as tc:
        with (
            tc.tile_pool(name="big", bufs=1) as big,
            tc.tile_pool(name="wrk", bufs=3) as wrk,
            tc.tile_pool(name="mtp", bufs=4) as mtp,
            tc.tile_pool(name="ps", bufs=8, space="PSUM") as ps,
        ):
            # ---------------- input DMAs (SP queue, priority order) ----------------
            blob_sb = big.tile([128, FB], f16, tag="blob")
            nc.sync.dma_start(out=blob_sb[:], in_=blob_d[:])
            sidx_sb = big.tile([2 * W, NI], i16, tag="sidx")
            nc.sync.dma_start(out=sidx_sb[:], in_=sidx_d[:])
            cx_sb = big.tile([128, 3, HW], f16, tag="cx")
            for u in range(3):
                nc.sync.dma_start(out=cx_sb[:, u, :], in_=cx_d[u * 128:(u + 1) * 128, :])
            xn_sb = big.tile([128, 3, NPIX], f16, tag="xn")
            nc.sync.dma_start(out=xn_sb[:], in_=xn_d[:].rearrange("(u p) n -> p u n", p=128))
            xt_sb = big.tile([128, NCHUNK, C], f16, tag="xt")
            for hf in range(2):
                nc.sync.dma_start(
                    out=xt_sb[:, hf * 8:(hf + 1) * 8, :],
                    in_=xt_d[hf * 1024:(hf + 1) * 1024, :].rearrange("(t p) c -> p t c", p=128))

            nc.gpsimd.load_library(library_config.local_scatter)
            ident = big.tile([128, 128], f16, tag="ident")
            masks.make_identity(nc, ident[:])

            # ---------------- pooling (sum; /64 folded into wkt) on DVE, f16 -------
            ctx_p = big.tile([128, 3, S * S], f16, tag="ctxp")
            with nc.allow_low_precision(reason="f16 8-elem partial sums; |err|<<2e-2 gate"):
                for u in range(3):
                    p1 = wrk.tile([128, H * S], f16, tag="p1")
                    nc.vector.tensor_reduce(
                        out=p1[:], in_=cx_sb[:, u, :].rearrange("p (h bw j) -> p h bw j", bw=S, j=8),
                        axis=mybir.AxisListType.X, op=mybir.AluOpType.add)
                    ap2 = p1[:].rearrange("p (bh i bw) -> p bh i bw", bh=S, i=8)
                    ap2 = ap2.rearrange("p bh i bw -> p bh bw i")
                    nc.vector.tensor_reduce(
                        out=ctx_p[:, u, :], in_=ap2,
                        axis=mybir.AxisListType.X, op=mybir.AluOpType.add)

            # ---------------- kf = (Wk/64) @ ctx_p : [192, 49] f16 ----------------
            kf_sb = [big.tile([128, S * S], f16, tag="kf0", name="kf0"),
                     big.tile([64, S * S], f16, tag="kf1", name="kf1")]
            for dc, dw in ((0, 128), (1, 64)):
                kf_ps = ps.tile([dw, S * S], f32, tag="ps")
                for u in range(3):
                    nc.tensor.matmul(kf_ps[:], blob_sb[:, 768 + u * 192 + dc * 128: 768 + u * 192 + dc * 128 + dw],
                                     ctx_p[:, u, :], start=(u == 0), stop=(u == 2))
                nc.vector.tensor_copy(kf_sb[dc][:], kf_ps[:])

            # ---------------- G = Wq^T @ kf : [384, 49] f16 ----------------
            g_sb = big.tile([128, 3, S * S], f16, tag="g")
            for u in range(3):
                g_ps = ps.tile([128, S * S], f32, tag="ps")
                nc.tensor.matmul(g_ps[:], blob_sb[:, u * 128:(u + 1) * 128], kf_sb[0][:],
                                 start=True, stop=False)
                nc.tensor.matmul(g_ps[:], blob_sb[0:64, 384 + u * 128:384 + (u + 1) * 128], kf_sb[1][:],
                                 start=False, stop=True)
                nc.scalar.copy(g_sb[:, u, :], g_ps[:])

            # ---------------- pair pipeline ----------------
            expa = big.tile([S * S, NPIX], f16, tag="expa")
            m_sb = big.tile([128, NPAIR * 3, 112], f16, tag="m")
            out_sb = big.tile([128, 3, NPIX], f16, tag="out")
            wwdt1 = blob_sb[0:S * S, 1344:1344 + NI]

            GROUPS = [(0, 4), (4, 8), (8, 12), (12, 14)]
            gi = 0

            for p0 in range(NPAIR):
                if p0 % 2 == 0:
                    # logits + exp for 2 pairs (224 cols)
                    c0, c1 = p0 * 112, min(NPIX, (p0 + 2) * 112)
                    lg = ps.tile([S * S, 224], f32, tag="ps")
                    for u in range(3):
                        nc.tensor.matmul(lg[:, 0:c1 - c0], g_sb[:, u, :], xn_sb[:, u, c0:c1],
                                         start=(u == 0), stop=(u == 2))
                    nc.scalar.activation(expa[:, c0:c1], lg[:, 0:c1 - c0],
                                         mybir.ActivationFunctionType.Exp)

                dyn_ps = ps.tile([2 * W, NI], f32, tag="ps")
                nc.tensor.matmul(dyn_ps[:], expa[:, p0 * 112:(p0 + 1) * 112],
                                 wwdt1, start=True, stop=True)
                rec = wrk.tile([2 * W, 1], f32, tag="rec")
                nc.vector.reciprocal(rec[:], dyn_ps[:, 25:26])
                d16 = wrk.tile([2 * W, NI], f16, tag="d16")
                nc.vector.tensor_scalar_mul(d16[:], dyn_ps[:], rec[:])
                mt = mtp.tile([2 * W, MCOLS], f16, tag="mt")
                nc.gpsimd.local_scatter(mt[:], d16[:], sidx_sb[:],
                                        channels=2 * W, num_elems=MCOLS, num_idxs=NI)
                tp_ps = ps.tile([128, 3, 112], f16, tag="ps")
                for t3 in range(3):
                    nc.tensor.transpose(tp_ps[:, t3, :], mt[:, t3 * 128:(t3 + 1) * 128],
                                        ident[0:112, 0:112])
                if p0 % 2 == 0:
                    nc.vector.tensor_copy(m_sb[:, p0 * 3:(p0 + 1) * 3, :], tp_ps[:])
                else:
                    nc.scalar.copy(m_sb[:, p0 * 3:(p0 + 1) * 3, :], tp_ps[:])

                # emit final matmul groups late enough that Ms are ready,
                # early enough to overlap: g0 after pair 7, g1 after 9, ...
                emit = {7: 0, 9: 1, 11: 2, 13: 3}.get(p0)
                if emit is not None:
                    ga, gb = GROUPS[emit]
                    npr = gb - ga
                    for cc in range(3):
                        po = ps.tile([128, npr * 112], f32, tag="ps")
                        for pp in range(ga, gb):
                            for trel in range(3):
                                nc.tensor.matmul(po[:, (pp - ga) * 112:(pp - ga + 1) * 112],
                                                 xt_sb[:, pp + trel, cc * 128:(cc + 1) * 128],
                                                 m_sb[:, pp * 3 + trel, :],
                                                 start=(trel == 0), stop=(trel == 2))
                        if cc % 2 == 0:
                            nc.vector.tensor_copy(out_sb[:, cc, ga * 112:gb * 112], po[:])
                        else:
                            nc.scalar.copy(out_sb[:, cc, ga * 112:gb * 112], po[:])
                    nc.sync.dma_start(
                        out=out_d[:].rearrange("(u p) n -> p u n", p=128)[:, :, ga * 112:gb * 112],
                        in_=out_sb[:, :, ga * 112:gb * 112])
                    gi += 1
    nc.finalize()
    return nc


def _static_inputs():
    # scatter index table: pixel p = hl*56 + w ; tap j = 5*di + dj
    sidx = np.full((2 * W, NI), -1, np.int16)
    for hl in range(2):
        for w in range(W):
            for di in range(KK):
                for dj in range(KK):
                    sidx[hl * W + w, 5 * di + dj] = (hl + di) * PADW + w + dj
    return sidx


def _prep(x, ctx, Wq, Wk, Wwd):
    sidx = _static_inputs()
    blob = np.zeros((128, FB), np.float16)
    blob[:, 0:384] = Wq[0:128, :]
    blob[0:64, 384:768] = Wq[128:192, :]
    wkt = (Wk.T / 64.0).astype(np.float16)          # [C, D2]
    for u in range(3):
        blob[:, 768 + u * 192:768 + (u + 1) * 192] = wkt[u * 128:(u + 1) * 128, :]
    blob[0:S * S, 1344:1344 + NI] = np.concatenate(
        [Wwd.T, np.ones((S * S, 1), np.float32)], axis=1)
    in_maps = []
    for core in range(NCORES):
        b, half = core // 2, core % 2
        r0 = half * ROWS
        xn = np.ascontiguousarray(x[b, :, r0:r0 + ROWS, :].reshape(C, NPIX)).astype(np.float16)
        xp = np.zeros((PADR, PADW, C), np.float32)
        lo, hi = max(0, r0 - 2), min(H, r0 + ROWS + 2)
        xp[lo - (r0 - 2):hi - (r0 - 2), 2:2 + W, :] = np.transpose(x[b, :, lo:hi, :], (1, 2, 0))
        xt = xp.reshape(WSPACE, C).astype(np.float16)
        cx = np.ascontiguousarray(ctx[b].reshape(C, HW)).astype(np.float16)
        in_maps.append(dict(blob=blob, sidx=sidx, cx=cx, xn=xn, xt=xt))
    return in_maps


def kernel(x, ctx, Wq, Wk, Wwd, _trace=False):
    from concourse.bass_utils import run_bass_kernel_spmd

    x, ctx = np.asarray(x), np.asarray(ctx)
    Wq, Wk, Wwd = np.asarray(Wq), np.asarray(Wk), np.asarray(Wwd)
    if "nc" not in _cached:
        _cached["nc"] = _build_nc()
    in_maps = _prep(x, ctx, Wq, Wk, Wwd)
    res = run_bass_kernel_spmd(_cached["nc"], in_maps, list(range(NCORES)), trace=_trace)
    _cached["last_result"] = res
    out = np.empty((B, C, H, W), np.float32)
    for core in range(NCORES):
        b, half = core // 2, core % 2
        r0 = half * ROWS
        out[b, :, r0:r0 + ROWS, :] = res.results[core]["out"].astype(np.float32).reshape(C, ROWS, W)
    return out


# revision 10
# speedup vs baseline: 3.4321x; 1.7317x over previous
"""ContMix kernel for TRN2, 8 NeuronCores.

Sharding: (batch b, H-half) -> 8 cores. Each core computes out[b, :, r0:r0+28, :].

Pipeline (per core):
  pooling: f16 DVE+gpsimd split reduce -> ctx_p [C, 49]
  kf = (Wk/64) @ ctx_p ; G = Wq^T @ kf        (f16 matmuls)
  per 224-col chunk: logits lg = G^T x, expa = exp(lg)  (f16, no max-sub)
  per pair: dyn = expa^T @ [Wwd^T|1] -> d16 = dyn/denom   (PE + DVE, decoupled)
  per 2 pairs: gpsimd local_scatter -> M^T [112, 768]; PE transpose -> M chunks
  final: banded matmuls xt^T @ M accumulated over 3 chunks; out streamed in 4 DMAs
All DMA traffic f16; weights packed into one blob DMA.
"""

import numpy as np

B, C, H, W = 4, 384, 56, 56
KK, S = 5, 7
NCORES = 8
ROWS = H // 2              # 28 rows per core
NPIX = ROWS * W            # 1568
PADR = ROWS + 4            # 32 padded rows
PADW = 64                  # padded width
WSPACE = PADR * PADW       # 2048 padded pixels
NPAIR = ROWS // 2          # 14 output row-pairs
NCHUNK = PADR // 2         # 16 contraction chunks
D2 = C // 2                # 192
NI = 26                    # scatter idxs (25 taps + 1 denom col)
MCOLS = 3 * 128            # 384 = w''-space per pair
HW = H * W                 # 3136
FB = 1370                  # blob cols: wqA 0:384, wqB 384:768, wkt 768:1344, wwdt1 1344:1370

_cached = {}


def _build_nc():
    import concourse.tile as tile
    from concourse import bacc, mybir, library_config, masks

    f32, f16, i16 = mybir.dt.float32, mybir.dt.float16, mybir.dt.int16
    nc = bacc.Bacc("TRN2", target_bir_lowering=False, debug=False)

    blob_d = nc.dram_tensor("blob", [128, FB], f16, kind="ExternalInput")
    sidx_d = nc.dram_tensor("sidx", [2 * W, 2 * NI], i16, kind="ExternalInput")
    cx_d = nc.dram_tensor("cx", [C, HW], f16, kind="ExternalInput")
    xn_d = nc.dram_tensor("xn", [C, NPIX], f16, kind="ExternalInput")
    xt_d = nc.dram_tensor("xt", [WSPACE, C], f16, kind="ExternalInput")
    out_d = nc.dram_tensor("out", [C, NPIX], f16, kind="ExternalOutput")

    with tile.TileContext(nc) as tc:
        with (
            tc.tile_pool(name="big", bufs=1) as big,
            tc.tile_pool(name="wrk", bufs=3) as wrk,
            tc.tile_pool(name="mtp", bufs=4) as mtp,
            tc.tile_pool(name="ps", bufs=8, space="PSUM") as ps,
        ):
            # ---------------- input DMAs (SP queue = transfer order) --------------
            sidx_sb = big.tile([2 * W, 2 * NI], i16, tag="sidx")
            nc.sync.dma_start(out=sidx_sb[:], in_=sidx_d[:])
            cx_sb = big.tile([128, 3, HW], f16, tag="cx")
            for u in range(2):
                nc.sync.dma_start(out=cx_sb[:, u, :], in_=cx_d[u * 128:(u + 1) * 128, :])
            blob_sb = big.tile([128, FB], f16, tag="blob")
            nc.sync.dma_start(out=blob_sb[:], in_=blob_d[:])
            nc.sync.dma_start(out=cx_sb[:, 2, 0:24 * W], in_=cx_d[256:384, 0:24 * W])
            nc.sync.dma_start(out=cx_sb[:, 2, 24 * W:], in_=cx_d[256:384, 24 * W:])
            xn_sb = big.tile([128, 3, NPIX], f16, tag="xn")
            nc.sync.dma_start(out=xn_sb[:], in_=xn_d[:].rearrange("(u p) n -> p u n", p=128))
            xt_sb = big.tile([128, NCHUNK, C], f16, tag="xt")
            for hf in range(2):
                nc.sync.dma_start(
                    out=xt_sb[:, hf * 8:(hf + 1) * 8, :],
                    in_=xt_d[hf * 1024:(hf + 1) * 1024, :].rearrange("(t p) c -> p t c", p=128))

            nc.gpsimd.load_library(library_config.local_scatter)
            ident = big.tile([128, 128], f16, tag="ident")
            masks.make_identity(nc, ident[:])

            # ------- pooling (sum; /64 folded into wkt): DVE f16 add-tree ---------
            # rows first (i=8 within bin-row), then cols (j=8 within bin-col);
            # adds keep innermost packed so DVE 2x f16 mode applies.
            add = mybir.AluOpType.add
            ctx_p = big.tile([128, 3, S * S], f16, tag="ctxp")
            with nc.allow_low_precision(reason="f16 partial sums; |err|<<2e-2 gate"):
                for u, b0, b1_ in ((0, 0, S), (1, 0, S), (2, 0, 3), (2, 3, S)):
                    nb = b1_ - b0
                    v = cx_sb[:, u, b0 * 8 * W:b1_ * 8 * W].rearrange(
                        "p (bh i w) -> p bh i w", i=8, w=W)
                    a1 = wrk.tile([128, S, 4, W], f16, tag="a1")
                    nc.vector.tensor_tensor(out=a1[:, 0:nb], in0=v[:, :, 0:4, :],
                                            in1=v[:, :, 4:8, :], op=add)
                    a2 = wrk.tile([128, S, 2, W], f16, tag="a2")
                    nc.vector.tensor_tensor(out=a2[:, 0:nb], in0=a1[:, 0:nb, 0:2, :],
                                            in1=a1[:, 0:nb, 2:4, :], op=add)
                    a3 = wrk.tile([128, S, 1, W], f16, tag="a3")
                    nc.vector.tensor_tensor(out=a3[:, 0:nb], in0=a2[:, 0:nb, 0:1, :],
                                            in1=a2[:, 0:nb, 1:2, :], op=add)
                    d = a3[:, 0:nb].rearrange("p bh one (bw j) -> p (bh one) bw j", bw=S, j=8)
                    b1 = wrk.tile([128, S, S, 4], f16, tag="b1")
                    nc.vector.tensor_tensor(out=b1[:, 0:nb], in0=d[:, :, :, 0:4],
                                            in1=d[:, :, :, 4:8], op=add)
                    b2 = wrk.tile([128, S, S, 2], f16, tag="b2")
                    nc.vector.tensor_tensor(out=b2[:, 0:nb], in0=b1[:, 0:nb, :, 0:2],
                                            in1=b1[:, 0:nb, :, 2:4], op=add)
                    nc.vector.tensor_reduce(
                        out=ctx_p[:, u, b0 * S:b1_ * S],
                        in_=b2[:, 0:nb].rearrange("p bh bw j -> p (bh bw) j"),
                        axis=mybir.AxisListType.X, op=add)

            # ---------------- kf = (Wk/64) @ ctx_p : [192, 49] f16 ----------------
            kf_sb = [big.tile([128, S * S], f16, tag="kf0", name="kf0"),
                     big.tile([64, S * S], f16, tag="kf1", name="kf1")]
            for dc, dw in ((0, 128), (1, 64)):
                kf_ps = ps.tile([dw, S * S], f32, tag="ps")
                for u in range(3):
                    nc.tensor.matmul(kf_ps[:], blob_sb[:, 768 + u * 192 + dc * 128: 768 + u * 192 + dc * 128 + dw],
                                     ctx_p[:, u, :], start=(u == 0), stop=(u == 2))
                if dc == 0:
                    nc.vector.tensor_copy(kf_sb[dc][:], kf_ps[:])
                else:
                    nc.scalar.copy(kf_sb[dc][:], kf_ps[:])

            # ---------------- G = Wq^T @ kf : [384, 49] f16 ----------------
            g_sb = big.tile([128, 3, S * S], f16, tag="g")
            for u in range(3):
                g_ps = ps.tile([128, S * S], f32, tag="ps")
                nc.tensor.matmul(g_ps[:], blob_sb[:, u * 128:(u + 1) * 128], kf_sb[0][:],
                                 start=True, stop=False)
                nc.tensor.matmul(g_ps[:], blob_sb[0:64, 384 + u * 128:384 + (u + 1) * 128], kf_sb[1][:],
                                 start=False, stop=True)
                if u == 1:
                    nc.scalar.copy(g_sb[:, u, :], g_ps[:])
                else:
                    nc.vector.tensor_copy(g_sb[:, u, :], g_ps[:])

            # ------- B1: logits + exp + dyn + normalize (d16 for all pairs) -------
            expa = big.tile([S * S, NPIX], f16, tag="expa")
            d16a = big.tile([2 * W, NPAIR * NI], f16, tag="d16a")
            wwdt1 = blob_sb[0:S * S, 1344:1344 + NI]
            for ch in range(7):
                c0, c1 = ch * 224, (ch + 1) * 224
                lg = ps.tile([S * S, 224], f32, tag="ps")
                for u in range(3):
                    nc.tensor.matmul(lg[:], g_sb[:, u, :], xn_sb[:, u, c0:c1],
                                     start=(u == 0), stop=(u == 2))
                nc.scalar.activation(expa[:, c0:c1], lg[:],
                                     mybir.ActivationFunctionType.Exp)
                for p0 in (2 * ch, 2 * ch + 1):
                    dyn_ps = ps.tile([2 * W, NI], f32, tag="ps")
                    nc.tensor.matmul(dyn_ps[:], expa[:, p0 * 112:(p0 + 1) * 112],
                                     wwdt1, start=True, stop=True)
                    rec = wrk.tile([2 * W, 1], f32, tag="rec")
                    nc.vector.reciprocal(rec[:], dyn_ps[:, 25:26])
                    nc.vector.tensor_scalar_mul(d16a[:, p0 * NI:(p0 + 1) * NI], dyn_ps[:], rec[:])

            # ------- B2/B3: scatter (2 pairs/instr), PE transpose, final matmuls ---
            m_sb = big.tile([128, NPAIR * 3, 112], f16, tag="m")
            out_sb = big.tile([128, 3, NPIX], f16, tag="out")
            DMAS = {1: (0, 448), 3: (448, 896), 5: (896, 1344), 6: (1344, 1568)}

            for sp in range(7):
                mt = mtp.tile([2 * W, 2 * MCOLS], f16, tag="mt")
                nc.gpsimd.local_scatter(mt[:], d16a[:, sp * 2 * NI:(sp + 1) * 2 * NI],
                                        sidx_sb[:], channels=2 * W,
                                        num_elems=2 * MCOLS, num_idxs=2 * NI)
                tp_ps = ps.tile([128, 6, 112], f16, tag="ps")
                for t6 in range(6):
                    nc.tensor.transpose(tp_ps[:, t6, :], mt[:, t6 * 128:(t6 + 1) * 128],
                                        ident[0:112, 0:112])
                if sp % 2 == 0:
                    nc.vector.tensor_copy(m_sb[:, sp * 6:(sp + 1) * 6, :], tp_ps[:])
                else:
                    nc.scalar.copy(m_sb[:, sp * 6:(sp + 1) * 6, :], tp_ps[:])

                ga, gb = 2 * sp, 2 * sp + 2
                for cc in range(3):
                    po = ps.tile([128, 224], f32, tag="ps")
                    for pp in range(ga, gb):
                        for trel in range(3):
                            nc.tensor.matmul(po[:, (pp - ga) * 112:(pp - ga + 1) * 112],
                                             xt_sb[:, pp + trel, cc * 128:(cc + 1) * 128],
                                             m_sb[:, pp * 3 + trel, :],
                                             start=(trel == 0), stop=(trel == 2))
                    if cc % 2 == 0:
                        nc.vector.tensor_copy(out_sb[:, cc, ga * 112:gb * 112], po[:])
                    else:
                        nc.scalar.copy(out_sb[:, cc, ga * 112:gb * 112], po[:])
                if sp in DMAS:
                    ca, cb = DMAS[sp]
                    nc.sync.dma_start(
                        out=out_d[:].rearrange("(u p) n -> p u n", p=128)[:, :, ca:cb],
                        in_=out_sb[:, :, ca:cb])
    nc.finalize()
    return nc


def _static_inputs():
    # scatter index table for TWO adjacent pairs: pixel p = hl*56 + w,
    # tap j = 5*di + dj; second pair's M^T lives at col offset MCOLS.
    sidx = np.full((2 * W, 2 * NI), -1, np.int16)
    for half in range(2):
        for hl in range(2):
            for w in range(W):
                for di in range(KK):
                    for dj in range(KK):
                        sidx[hl * W + w, half * NI + 5 * di + dj] = \
                            half * MCOLS + (hl + di) * PADW + w + dj
    return sidx


def _prep(x, ctx, Wq, Wk, Wwd):
    sidx = _static_inputs()
    blob = np.zeros((128, FB), np.float16)
    blob[:, 0:384] = Wq[0:128, :]
    blob[0:64, 384:768] = Wq[128:192, :]
    wkt = (Wk.T / 64.0).astype(np.float16)          # [C, D2]
    for u in range(3):
        blob[:, 768 + u * 192:768 + (u + 1) * 192] = wkt[u * 128:(u + 1) * 128, :]
    blob[0:S * S, 1344:1344 + NI] = np.concatenate(
        [Wwd.T, np.ones((S * S, 1), np.float32)], axis=1)
    in_maps = []
    for core in range(NCORES):
        b, half = core // 2, core % 2
        r0 = half * ROWS
        xn = np.ascontiguousarray(x[b, :, r0:r0 + ROWS, :].reshape(C, NPIX)).astype(np.float16)
        xp = np.zeros((PADR, PADW, C), np.float32)
        lo, hi = max(0, r0 - 2), min(H, r0 + ROWS + 2)
        xp[lo - (r0 - 2):hi - (r0 - 2), 2:2 + W, :] = np.transpose(x[b, :, lo:hi, :], (1, 2, 0))
        xt = xp.reshape(WSPACE, C).astype(np.float16)
        cx = np.ascontiguousarray(ctx[b].reshape(C, HW)).astype(np.float16)
        in_maps.append(dict(blob=blob, sidx=sidx, cx=cx, xn=xn, xt=xt))
    return in_maps


def kernel(x, ctx, Wq, Wk, Wwd, _trace=False):
    from concourse.bass_utils import run_bass_kernel_spmd

    x, ctx = np.asarray(x), np.asarray(ctx)
    Wq, Wk, Wwd = np.asarray(Wq), np.asarray(Wk), np.asarray(Wwd)
    if "nc" not in _cached:
        _cached["nc"] = _build_nc()
    in_maps = _prep(x, ctx, Wq, Wk, Wwd)
    res = run_bass_kernel_spmd(_cached["nc"], in_maps, list(range(NCORES)), trace=_trace)
    _cached["last_result"] = res
    out = np.empty((B, C, H, W), np.float32)
    for core in range(NCORES):
        b, half = core // 2, core % 2
        r0 = half * ROWS
        out[b, :, r0:r0 + ROWS, :] = res.results[core]["out"].astype(np.float32).reshape(C, ROWS, W)
    return out
